# revision 10
# baseline (speedup 1.0000x reference)
"""Trainium2 Bass kernel for nn_ContextualModel_75806172774985.

Per-sample computation (B = 4M samples, S=4 steps, Q=5 features):
    y[b, m] = sum_{s < L[b]} q0[b,s] * (A @ feats[b,s])[m],
    A = W_reg @ W_kernel  (4x4)

Memory-bound problem; strategy:
  - Host converts inputs to fp16 and lays them out as dense per-partition
    streams (s-major), halving HBM traffic and making every device op a
    unit-stride fp16 op (DVE 2x/4x fast modes):
        q0sm [P, 4, T]   q0sm[p, s, t]   = xss[p*T+t, s, 0]
        fsf  [P, 16, T]  fsf[p, 4s+f, t] = xss[p*T+t, s, 1+f]
        lst  [P, T]      seq_lengths as fp16
    Output y [P, 4, T] fp16 m-major; host transposes back and upcasts.
  - Pure data parallel across 8 cores (500k samples each, zero-padded to
    507904 = 128 * 3968).
  - No TensorE/PSUM in the main loop. Each tile's sample columns are split
    between DVE (fast) and GpSimd (slow) so both engines run the identical
    independent program on disjoint columns: zero cross-engine edges.
  - Per engine, per tile (cols = its sample slice):
        Z[s]   = (L > s) * q0[s]          4x scalar_tensor_tensor
        acc[f] = sum_s Z[s] * f[4s+f]     16 muls + 3 slab adds
        y[m]   = sum_f A[m,f] * acc[f]    16 MACs (stt with AP scalar)
    A is computed on device once and broadcast to all partitions with a
    tiny ones-matmul on TensorE.
"""
import numpy as np
from concurrent.futures import ThreadPoolExecutor

import concourse.bass as bass
import concourse.tile as tile
from concourse import bacc, mybir
from concourse.bass_utils import run_bass_kernel_spmd

N_CORES = 8
P = 128
B_TOTAL = 4_000_000
BS = B_TOTAL // N_CORES          # 500_000 samples per core
T = 3968                         # samples per partition (128*3968 = 507904)
BSP = P * T

f32 = mybir.dt.float32
f16 = mybir.dt.float16

K_TILES = (1488, 1488, 992)      # sample columns per tile
DVE_FRAC = 0.81                  # fraction of each tile's columns on DVE


def _split(k):
    k1 = int(k * DVE_FRAC + 0.5)
    return k1, k - k1


def build_nc(num_devices=N_CORES):
    assert sum(K_TILES) == T
    nc = bacc.Bacc("TRN2", target_bir_lowering=False, debug=False,
                   enable_asserts=False, num_devices=num_devices)

    q0_d = nc.dram_tensor("q0", [P, 4 * T], f16, kind="ExternalInput")
    f_d = nc.dram_tensor("fsf", [P, 16 * T], f16, kind="ExternalInput")
    l_d = nc.dram_tensor("lst", [P, T], f16, kind="ExternalInput")
    wk_d = nc.dram_tensor("w_kernel", [4, 4], f32, kind="ExternalInput")
    wr_d = nc.dram_tensor("w_reg", [4, 4], f32, kind="ExternalInput")
    y_d = nc.dram_tensor("y", [P, 4 * T], f16, kind="ExternalOutput")

    ones_np = np.ones((4, 128), dtype=np.float32)
    ones_d = nc.inline_tensor(ones_np, name="ones4x128")

    q0_h = q0_d.ap().rearrange("p (s t) -> p s t", s=4)      # [128, 4, T]
    f_h = f_d.ap().rearrange("p (j t) -> p j t", j=16)       # [128, 16, T]
    l_h = l_d.ap()                                           # [128, T]
    y_h = y_d.ap().rearrange("p (m t) -> p m t", m=4)        # [128, 4, T]

    mult = mybir.AluOpType.mult
    add = mybir.AluOpType.add
    is_gt = mybir.AluOpType.is_gt

    with tile.TileContext(nc) as tc:
        with (
            tc.tile_pool(name="inq", bufs=2) as inq_pool,
            tc.tile_pool(name="inf", bufs=2) as inf_pool,
            tc.tile_pool(name="inl", bufs=2) as inl_pool,
            tc.tile_pool(name="work", bufs=1) as work,
            tc.tile_pool(name="yout", bufs=2) as y_pool,
            tc.tile_pool(name="singles", bufs=1) as singles,
            tc.tile_pool(name="ps", bufs=1, space="PSUM") as ps_pool,
        ):
            # ---- one-time: a_sb[p, 4f+m] = A[m,f] = sum_c Wreg[m,c] Wkern[c,f]
            wrT = singles.tile([4, 4], f32)
            nc.sync.dma_start(out=wrT[:], in_=wr_d.ap().transpose([1, 0]))
            wk_s = singles.tile([4, 4], f32)
            nc.sync.dma_start(out=wk_s[:], in_=wk_d.ap())
            ones_sb = singles.tile([4, 128], f32)
            nc.sync.dma_start(out=ones_sb[:], in_=ones_d.ap())

            # R[c, (f,m)] = WregT[c, m] * Wkern[c, f]
            wrT_rep = bass.AP(tensor=wrT.tensor, offset=wrT.offset,
                              ap=[list(wrT.ap[0]), [0, 4], [1, 4]])
            wk_rep = bass.AP(tensor=wk_s.tensor, offset=wk_s.offset,
                             ap=[list(wk_s.ap[0]), [1, 4], [0, 4]])
            r_sb = singles.tile([4, 4, 4], f32)
            nc.vector.tensor_tensor(out=r_sb[:], in0=wrT_rep, in1=wk_rep,
                                    op=mult)
            a_ps = ps_pool.tile([128, 16], f32)
            nc.tensor.matmul(a_ps[:], ones_sb[:],
                             r_sb.rearrange("c f m -> c (f m)"))
            a_sb = singles.tile([128, 16], f32)
            nc.scalar.copy(a_sb[:], a_ps[:])

            # ---- main loop ----
            k1max = _split(max(K_TILES))[0]
            k2max = max(K_TILES) - k1max
            wt = {}
            for tg, kk in (("v", k1max), ("g", k2max)):
                z_w = work.tile([P, 4, kk], f16, tag="z" + tg)
                m_w = work.tile([P, 4, kk], f16, tag="m" + tg)
                a_w = work.tile([P, 4, kk], f16, tag="a" + tg)
                wt[tg] = (z_w, m_w, a_w)

            base = 0
            for K in K_TILES:
                k1, _ = _split(K)
                q0t = inq_pool.tile([P, 4, K], f16)
                nc.sync.dma_start(out=q0t[:], in_=q0_h[:, :, base:base + K])
                ft = inf_pool.tile([P, 16, K], f16)
                nc.sync.dma_start(out=ft[:], in_=f_h[:, :, base:base + K])
                lt = inl_pool.tile([P, K], f16)
                nc.sync.dma_start(out=lt[:], in_=l_h[:, base:base + K])

                yt = y_pool.tile([P, 4, K], f16)

                for eng, tg, sl, kk in ((nc.vector, "v", slice(0, k1), k1),
                                        (nc.gpsimd, "g", slice(k1, K),
                                         K - k1)):
                    zf, m4f, accf = wt[tg]
                    z = zf[:, :, :kk]
                    m4 = m4f[:, :, :kk]
                    acc = accf[:, :, :kk]
                    is_dve = tg == "v"

                    # Z[s] = (L > s) * q0[s]
                    if is_dve:
                        for s in range(4):
                            eng.scalar_tensor_tensor(
                                out=z[:, s, :], in0=lt[:, sl],
                                scalar=float(s), in1=q0t[:, s, sl],
                                op0=is_gt, op1=mult)
                    else:
                        for s in range(4):
                            eng.tensor_scalar(out=z[:, s, :], in0=lt[:, sl],
                                              scalar1=float(s), scalar2=None,
                                              op0=is_gt)
                        eng.tensor_tensor(out=z, in0=z, in1=q0t[:, :, sl],
                                          op=mult)
                    # acc[f] = sum_s Z[s] * feats[4s+f]
                    for s in range(4):
                        dst = acc if s == 0 else m4
                        for f in range(4):
                            eng.tensor_tensor(
                                out=dst[:, f, :], in0=z[:, s, :],
                                in1=ft[:, 4 * s + f, sl], op=mult)
                        if s > 0:
                            eng.tensor_tensor(out=acc, in0=acc,
                                              in1=m4, op=add)
                    # y[m] = sum_f A[m,f] * acc[f]
                    for m in range(4):
                        if is_dve:
                            for f in range(4):
                                if f == 0:
                                    eng.scalar_tensor_tensor(
                                        out=yt[:, m, sl], in0=acc[:, f, :],
                                        scalar=a_sb[:, m:m + 1],
                                        in1=acc[:, f, :], op0=mult,
                                        op1=mybir.AluOpType.bypass)
                                else:
                                    eng.scalar_tensor_tensor(
                                        out=yt[:, m, sl], in0=acc[:, f, :],
                                        scalar=a_sb[:, 4 * f + m:4 * f + m + 1],
                                        in1=yt[:, m, sl], op0=mult, op1=add)
                        else:
                            # t[f] = A[m,f] * acc[f]; y[m] = tree-sum
                            for f in range(4):
                                eng.tensor_scalar(
                                    out=m4[:, f, :], in0=acc[:, f, :],
                                    scalar1=a_sb[:, 4 * f + m:4 * f + m + 1],
                                    scalar2=None, op0=mult)
                            eng.tensor_tensor(out=m4[:, 0, :],
                                              in0=m4[:, 0, :],
                                              in1=m4[:, 1, :], op=add)
                            eng.tensor_tensor(out=m4[:, 2, :],
                                              in0=m4[:, 2, :],
                                              in1=m4[:, 3, :], op=add)
                            eng.tensor_tensor(out=yt[:, m, sl],
                                              in0=m4[:, 0, :],
                                              in1=m4[:, 2, :], op=add)

                nc.sync.dma_start(out=y_h[:, :, base:base + K], in_=yt[:])
                base += K
    nc.compile()
    return nc


_NC_CACHE = None


def _get_nc():
    global _NC_CACHE
    if _NC_CACHE is None:
        _NC_CACHE = build_nc()
    return _NC_CACHE


def _prep_core(args):
    xss, seq, c = args
    x = np.zeros((BSP, 4, 5), np.float16)
    x[:BS] = xss[c * BS:(c + 1) * BS]
    lp = np.zeros((BSP,), np.float16)
    lp[:BS] = seq[c * BS:(c + 1) * BS]
    arr = x.reshape(P, T, 4, 5)
    q0sm = np.ascontiguousarray(arr[:, :, :, 0].transpose(0, 2, 1))
    fsf = np.ascontiguousarray(
        arr[:, :, :, 1:].transpose(0, 2, 3, 1)).reshape(P, 16 * T)
    return {
        "q0": q0sm.reshape(P, 4 * T),
        "fsf": fsf,
        "lst": lp.reshape(P, T),
    }


def _shard_inputs(xss, seq_lengths, W_kernel, W_reg):
    xss = np.asarray(xss, dtype=np.float32).reshape(B_TOTAL, 4, 5)
    seq = np.asarray(seq_lengths)
    wk = np.ascontiguousarray(W_kernel, dtype=np.float32)
    wr = np.ascontiguousarray(W_reg, dtype=np.float32)
    with ThreadPoolExecutor(N_CORES) as ex:
        maps = list(ex.map(_prep_core,
                           [(xss, seq, c) for c in range(N_CORES)]))
    for m in maps:
        m["w_kernel"] = wk
        m["w_reg"] = wr
    return maps


def run(xss, seq_lengths, W_kernel, W_reg, trace=False, **spmd_kwargs):
    nc = _get_nc()
    in_maps = _shard_inputs(xss, seq_lengths, W_kernel, W_reg)
    res = run_bass_kernel_spmd(nc, in_maps, core_ids=list(range(N_CORES)),
                               trace=trace, **spmd_kwargs)

    def _post(r):
        y = r["y"].reshape(P, 4, T).transpose(0, 2, 1).reshape(BSP, 4)
        return y[:BS].astype(np.float32)

    with ThreadPoolExecutor(N_CORES) as ex:
        parts = list(ex.map(_post, res.results))
    out = np.concatenate(parts, axis=0)
    return out, res


def kernel(xss, seq_lengths, W_kernel, W_reg):
    out, _ = run(xss, seq_lengths, W_kernel, W_reg)
    return out


# revision 15
# speedup vs baseline: 1.3756x; 1.3756x over previous
"""Trainium2 Bass kernel for nn_ContextualModel_75806172774985.

Per-sample computation (B = 4M samples, S=4 steps, Q=5 features):
    y[b, m] = sum_{s < L[b]} q0[b,s] * (A @ feats[b,s])[m],
    A = W_reg @ W_kernel  (4x4)

Memory-bound problem; strategy:
  - Host converts inputs to fp16 and packs one dense per-partition stream
    (s-major), halving HBM traffic and making every device op a unit-stride
    fp16 slab op:
        cin [P, 21, T]: rows 0-3  q0[p*T+t, s]
                        rows 4-19 feats[p*T+t, s, f] (row 4+4s+f)
                        row  20   seq_lengths (fp16)
    Output y [P, 4, T] fp16 m-major; host transposes back and upcasts.
  - Pure data parallel across 8 cores (500k samples each, zero-padded to
    507904 = 128 * 3968).
  - No TensorE/PSUM in the main loop. Each tile's sample columns are split
    between DVE and GpSimd; both run the identical 8-instruction slab
    program on disjoint columns: zero cross-engine edges.
  - Per engine, per tile (kk = its column count), all tensor_tensor:
        zm = (L > srep)            [P,4,kk]   bcast L over s
        z  = zm * q0               [P,4,kk]
        M  = z(bcast f) * feats    [P,16,kk]
        M[0:2] += M[2:4]           8 rows     (in-place s-pair add)
        c  = M[0] + M[1]           [P,4,kk]
        t  = c(bcast m) * arep     [P,16,kk]  (t aliases M)
        t[:, :, 0:2] += t[:, :, 2:4]  8 rows  (in-place f-pair add)
        y  = t[:,:,0] + t[:,:,1]   [P,4,kk]
    arep[p, 4m+f, :] = A[m,f] is materialized once (Act engine, overlapped
    with the first input DMA); A itself is computed on device with one
    tiny ones-matmul broadcast.
"""
import numpy as np
from concurrent.futures import ThreadPoolExecutor

import concourse.bass as bass
import concourse.tile as tile
from concourse import bacc, mybir
from concourse.bass_utils import run_bass_kernel_spmd

N_CORES = 8
P = 128
B_TOTAL = 4_000_000
BS = B_TOTAL // N_CORES          # 500_000 samples per core
T = 3968                         # samples per partition (128*3968 = 507904)
BSP = P * T

f32 = mybir.dt.float32
f16 = mybir.dt.float16

K_TILES = (992, 992, 992, 992)   # sample columns per tile
DVE_FRAC = 0.56                 # fraction of each tile's columns on DVE


def _split(k):
    k1 = int(k * DVE_FRAC + 0.5)
    return k1, k - k1


def build_nc(num_devices=N_CORES):
    assert sum(K_TILES) == T
    nc = bacc.Bacc("TRN2", target_bir_lowering=False, debug=False,
                   enable_asserts=False, num_devices=num_devices)

    c_d = nc.dram_tensor("cin", [P, 21 * T], f16, kind="ExternalInput")
    wk_d = nc.dram_tensor("w_kernel", [4, 4], f32, kind="ExternalInput")
    wr_d = nc.dram_tensor("w_reg", [4, 4], f32, kind="ExternalInput")
    y_d = nc.dram_tensor("y", [P, 4 * T], f16, kind="ExternalOutput")

    ones_np = np.ones((4, 128), dtype=np.float32)
    ones_d = nc.inline_tensor(ones_np, name="ones4x128")

    c_h = c_d.ap().rearrange("p (r t) -> p r t", r=21)       # [128, 21, T]
    y_h = y_d.ap().rearrange("p (m t) -> p m t", m=4)        # [128, 4, T]

    mult = mybir.AluOpType.mult
    add = mybir.AluOpType.add
    is_gt = mybir.AluOpType.is_gt

    k1max, k2max = _split(max(K_TILES))
    kemax = {"v": k1max, "g": k2max}

    with tile.TileContext(nc) as tc:
        with (
            tc.tile_pool(name="cin", bufs=2) as in_pool,
            tc.tile_pool(name="work", bufs=1) as work,
            tc.tile_pool(name="zp", bufs=2) as z_pool,
            tc.tile_pool(name="yout", bufs=2) as y_pool,
            tc.tile_pool(name="singles", bufs=1) as singles,
            tc.tile_pool(name="ps", bufs=1, space="PSUM") as ps_pool,
        ):
            # ---- one-time: a_sb[p, 4f+m] = A[m,f] = sum_c Wreg[m,c] Wkern[c,f]
            wrT = singles.tile([4, 4], f32)
            nc.sync.dma_start(out=wrT[:], in_=wr_d.ap().transpose([1, 0]))
            wk_s = singles.tile([4, 4], f32)
            nc.sync.dma_start(out=wk_s[:], in_=wk_d.ap())
            ones_sb = singles.tile([4, 128], f32)
            nc.sync.dma_start(out=ones_sb[:], in_=ones_d.ap())

            # R[c, (f,m)] = WregT[c, m] * Wkern[c, f]
            wrT_rep = bass.AP(tensor=wrT.tensor, offset=wrT.offset,
                              ap=[list(wrT.ap[0]), [0, 4], [1, 4]])
            wk_rep = bass.AP(tensor=wk_s.tensor, offset=wk_s.offset,
                             ap=[list(wk_s.ap[0]), [1, 4], [0, 4]])
            r_sb = singles.tile([4, 4, 4], f32)
            nc.vector.tensor_tensor(out=r_sb[:], in0=wrT_rep, in1=wk_rep,
                                    op=mult)
            a_ps = ps_pool.tile([128, 16], f32)
            nc.tensor.matmul(a_ps[:], ones_sb[:],
                             r_sb.rearrange("c f m -> c (f m)"))
            a_sb = singles.tile([128, 16], f32)
            nc.scalar.copy(a_sb[:], a_ps[:])

            # ---- one-time: srep, ones row, arep (A broadcast along cols)
            srep = singles.tile([P, 4, max(K_TILES)], f16)
            for s in range(4):
                nc.vector.memset(srep[:, s, :], float(s))
            onesk = singles.tile([P, k1max], f16)
            nc.vector.memset(onesk[:], 1.0)
            arep = singles.tile([P, 16, k1max], f16)
            for m in range(4):
                for f in range(4):
                    nc.scalar.activation(
                        out=arep[:, 4 * m + f, :], in_=onesk[:],
                        func=mybir.ActivationFunctionType.Copy,
                        scale=a_sb[:, 4 * f + m:4 * f + m + 1])

            # ---- per-engine work tiles ----
            wt = {}
            for tg in ("v", "g"):
                ke = kemax[tg]
                m_w = work.tile([P, 16, ke], f16, tag="m" + tg)
                c_w = work.tile([P, 4, ke], f16, tag="c" + tg)
                wt[tg] = (m_w, c_w)

            # ---- main loop ----
            base = 0
            for K in K_TILES:
                k1, _ = _split(K)
                ct = in_pool.tile([P, 21, K], f16)
                nc.sync.dma_start(out=ct[:], in_=c_h[:, :, base:base + K])
                yt = y_pool.tile([P, 4, K], f16)

                # z = (L > s) * q0 : full width on DVE (GpSimd cannot is_gt)
                z_sh = z_pool.tile([P, 4, K], f16, tag="zsh")
                zK = z_sh[:, :, :K]
                lbc = ct[:, 20:21, :].broadcast_to([P, 4, K])
                nc.vector.tensor_tensor(out=zK, in0=lbc,
                                        in1=srep[:, :, :K], op=is_gt)
                nc.vector.tensor_tensor(out=zK, in0=zK, in1=ct[:, 0:4, :],
                                        op=mult)

                for eng, tg, sl, kk in ((nc.vector, "v", slice(0, k1), k1),
                                        (nc.gpsimd, "g", slice(k1, K),
                                         K - k1)):
                    mf, cf = wt[tg]
                    c = cf[:, :, :kk]
                    msf = mf.rearrange("p (s f) k -> p s f k", s=4)
                    m_s = msf[:, :, :, :kk]
                    tmf = mf.rearrange("p (m f) k -> p m f k", m=4)
                    t_m = tmf[:, :, :, :kk]

                    fts = ct[:, 4:20, sl].rearrange("p (s f) k -> p s f k",
                                                    s=4)

                    # M[s,f] = z[s] * feats[s,f]
                    zbc = z_sh[:, :, sl].unsqueeze(2).broadcast_to(
                        [P, 4, 4, kk])
                    eng.tensor_tensor(out=m_s, in0=zbc, in1=fts, op=mult)
                    # c[f] = sum_s M[s,f]
                    eng.tensor_tensor(out=m_s[:, 0:2], in0=m_s[:, 0:2],
                                      in1=m_s[:, 2:4], op=add)
                    eng.tensor_tensor(out=c, in0=m_s[:, 0], in1=m_s[:, 1],
                                      op=add)
                    # t[m,f] = c[f] * A[m,f]  (t aliases M)
                    cbc = c.unsqueeze(1).broadcast_to([P, 4, 4, kk])
                    eng.tensor_tensor(out=t_m, in0=cbc,
                                      in1=arep[:, :, :kk].rearrange(
                                          "p (m f) k -> p m f k", m=4),
                                      op=mult)
                    # y[m] = sum_f t[m,f]
                    eng.tensor_tensor(out=t_m[:, :, 0:2], in0=t_m[:, :, 0:2],
                                      in1=t_m[:, :, 2:4], op=add)
                    eng.tensor_tensor(out=yt[:, :, sl], in0=t_m[:, :, 0],
                                      in1=t_m[:, :, 1], op=add)

                nc.sync.dma_start(out=y_h[:, :, base:base + K], in_=yt[:])
                base += K
    nc.compile()
    return nc


_NC_CACHE = None


def _get_nc():
    global _NC_CACHE
    if _NC_CACHE is None:
        _NC_CACHE = build_nc()
    return _NC_CACHE


def _prep_core(args):
    xss, seq, c = args
    x = np.zeros((BSP, 4, 5), np.float16)
    x[:BS] = xss[c * BS:(c + 1) * BS]
    lp = np.zeros((BSP,), np.float16)
    lp[:BS] = seq[c * BS:(c + 1) * BS]
    arr = x.reshape(P, T, 4, 5)
    cin = np.empty((P, 21, T), np.float16)
    cin[:, 0:4] = arr[:, :, :, 0].transpose(0, 2, 1)
    cin[:, 4:20] = arr[:, :, :, 1:].transpose(0, 2, 3, 1).reshape(P, 16, T)
    cin[:, 20] = lp.reshape(P, T)
    return {"cin": cin.reshape(P, 21 * T)}


def _shard_inputs(xss, seq_lengths, W_kernel, W_reg):
    xss = np.asarray(xss, dtype=np.float32).reshape(B_TOTAL, 4, 5)
    seq = np.asarray(seq_lengths)
    wk = np.ascontiguousarray(W_kernel, dtype=np.float32)
    wr = np.ascontiguousarray(W_reg, dtype=np.float32)
    with ThreadPoolExecutor(N_CORES) as ex:
        maps = list(ex.map(_prep_core,
                           [(xss, seq, c) for c in range(N_CORES)]))
    for m in maps:
        m["w_kernel"] = wk
        m["w_reg"] = wr
    return maps


def run(xss, seq_lengths, W_kernel, W_reg, trace=False, **spmd_kwargs):
    nc = _get_nc()
    in_maps = _shard_inputs(xss, seq_lengths, W_kernel, W_reg)
    res = run_bass_kernel_spmd(nc, in_maps, core_ids=list(range(N_CORES)),
                               trace=trace, **spmd_kwargs)

    def _post(r):
        y = r["y"].reshape(P, 4, T).transpose(0, 2, 1).reshape(BSP, 4)
        return y[:BS].astype(np.float32)

    with ThreadPoolExecutor(N_CORES) as ex:
        parts = list(ex.map(_post, res.results))
    out = np.concatenate(parts, axis=0)
    return out, res


def kernel(xss, seq_lengths, W_kernel, W_reg):
    out, _ = run(xss, seq_lengths, W_kernel, W_reg)
    return out


# revision 16
# speedup vs baseline: 3.0209x; 2.1960x over previous
"""Trainium2 Bass kernel for nn_ContextualModel_75806172774985.

Per-sample computation (B = 4M samples, S=4 steps, Q=5 features):
    y[b, m] = sum_{s < L[b]} q0[b,s] * (A @ feats[b,s])[m],
    A = W_reg @ W_kernel  (4x4)

Memory-bound problem. Measured engine rates (fp16, per elem per partition):
DVE tensor_tensor 0.54ns (any AP shape), tensor_scalar 0.31ns,
GpSimd 1.69ns, Act 0.93ns; engines contend heavily when run concurrently,
so the design keeps DVE as the single SBUF-elementwise engine and moves
the s-summation to TensorE (PSUM traffic, not SBUF).

  - Host converts inputs to fp16, packs one dense per-partition stream:
        cin [P, 21, T]: rows 0-3  q0, rows 4-19 feats (row 4+4s+f),
                        row 20 seq_lengths. Output y [P, 4, T] fp16
        m-major; host transposes back / upcasts.
  - Per tile (software-pipelined one deep):
        DVE : zm[s] = (L > s)        4x tensor_scalar (imm)
              z    = zm * q0         1x tensor_tensor
              M[4s+f] = z[s]*f[s,f]  1x tensor_tensor (bcast over f)
        PE  : c = sum_s M[4s:4s+4]   4 accumulating fp16 identity matmuls
              per 128-sample group -> PSUM (f-major [4,128] per group)
        Act : c PSUM -> SBUF fp16 copy per group
        DVE : t[4m+f] = A[m,f]*c[f]  16x tensor_scalar (AP scalar)
              y[m] = sum_f t[4m+f]   2x slab tensor_tensor adds
    A is computed on device once (tiny ones-matmul broadcast into a_sb).
"""
import numpy as np
from concurrent.futures import ThreadPoolExecutor

import concourse.bass as bass
import concourse.tile as tile
from concourse import bacc, mybir
from concourse.bass_utils import run_bass_kernel_spmd

N_CORES = 8
P = 128
B_TOTAL = 4_000_000
BS = B_TOTAL // N_CORES          # 500_000 samples per core
T = 3968                         # samples per partition (128*3968 = 507904)
BSP = P * T
GRP = 128                        # samples per PSUM group (512 psum cols)

f32 = mybir.dt.float32
f16 = mybir.dt.float16

K_TILES = (896, 896, 896, 896, 384)


def build_nc(num_devices=N_CORES):
    assert sum(K_TILES) == T
    for k in K_TILES:
        assert k % GRP == 0
    nc = bacc.Bacc("TRN2", target_bir_lowering=False, debug=False,
                   enable_asserts=False, num_devices=num_devices)

    c_d = nc.dram_tensor("cin", [P, 21 * T], f16, kind="ExternalInput")
    wk_d = nc.dram_tensor("w_kernel", [4, 4], f32, kind="ExternalInput")
    wr_d = nc.dram_tensor("w_reg", [4, 4], f32, kind="ExternalInput")
    y_d = nc.dram_tensor("y", [P, 4 * T], f16, kind="ExternalOutput")

    ones_np = np.ones((4, 128), dtype=np.float32)
    ones_d = nc.inline_tensor(ones_np, name="ones4x128")
    ident_np = np.eye(128, dtype=np.float16)
    ident_d = nc.inline_tensor(ident_np, name="ident128f16")

    c_h = c_d.ap().rearrange("p (r t) -> p r t", r=21)       # [128, 21, T]
    y_h = y_d.ap().rearrange("p (m t) -> p m t", m=4)        # [128, 4, T]

    mult = mybir.AluOpType.mult
    add = mybir.AluOpType.add
    is_gt = mybir.AluOpType.is_gt

    with tile.TileContext(nc) as tc:
        with (
            tc.tile_pool(name="cin", bufs=2) as in_pool,
            tc.tile_pool(name="mp", bufs=2) as m_pool,
            tc.tile_pool(name="cp", bufs=2) as c_pool,
            tc.tile_pool(name="work", bufs=1) as work,
            tc.tile_pool(name="yout", bufs=2) as y_pool,
            tc.tile_pool(name="singles", bufs=1) as singles,
            tc.tile_pool(name="ps", bufs=1, space="PSUM") as ps_pool,
            tc.tile_pool(name="psg", bufs=4, space="PSUM") as psg_pool,
        ):
            # ---- one-time: a_sb[p, 4f+m] = A[m,f] = sum_c Wreg[m,c]Wkern[c,f]
            wrT = singles.tile([4, 4], f32)
            nc.sync.dma_start(out=wrT[:], in_=wr_d.ap().transpose([1, 0]))
            wk_s = singles.tile([4, 4], f32)
            nc.sync.dma_start(out=wk_s[:], in_=wk_d.ap())
            ones_sb = singles.tile([4, 128], f32)
            nc.sync.dma_start(out=ones_sb[:], in_=ones_d.ap())
            ident = singles.tile([128, 128], f16)
            nc.sync.dma_start(out=ident[:], in_=ident_d.ap())

            wrT_rep = bass.AP(tensor=wrT.tensor, offset=wrT.offset,
                              ap=[list(wrT.ap[0]), [0, 4], [1, 4]])
            wk_rep = bass.AP(tensor=wk_s.tensor, offset=wk_s.offset,
                             ap=[list(wk_s.ap[0]), [1, 4], [0, 4]])
            r_sb = singles.tile([4, 4, 4], f32)
            nc.vector.tensor_tensor(out=r_sb[:], in0=wrT_rep, in1=wk_rep,
                                    op=mult)
            a_ps = ps_pool.tile([128, 16], f32)
            nc.tensor.matmul(a_ps[:], ones_sb[:],
                             r_sb.rearrange("c f m -> c (f m)"))
            a_sb = singles.tile([128, 16], f32)
            nc.scalar.copy(a_sb[:], a_ps[:])

            # ---- work tiles ----
            kmax = max(K_TILES)
            z_w = work.tile([P, 4, kmax], f16)
            t_w = work.tile([P, 16, kmax], f16)

            # ---- main loop, software-pipelined one tile deep ----
            pend = None              # (c_sb, yt, K, base) awaiting A-stage

            def a_stage(c_sb, yt, K):
                tk = t_w.rearrange("p (m f) k -> p m f k", m=4)[:, :, :, :K]
                for m in range(4):
                    for f in range(4):
                        nc.vector.tensor_scalar(
                            out=tk[:, m, f, :], in0=c_sb[:, f, :K],
                            scalar1=a_sb[:, 4 * f + m:4 * f + m + 1],
                            scalar2=None, op0=mult)
                nc.vector.tensor_tensor(out=tk[:, :, 0:2], in0=tk[:, :, 0:2],
                                        in1=tk[:, :, 2:4], op=add)
                nc.vector.tensor_tensor(out=yt[:], in0=tk[:, :, 0],
                                        in1=tk[:, :, 1], op=add)

            base = 0
            for K in K_TILES:
                ct = in_pool.tile([P, 21, K], f16)
                nc.sync.dma_start(out=ct[:], in_=c_h[:, :, base:base + K])
                yt = y_pool.tile([P, 4, K], f16)
                mt = m_pool.tile([P, 16, K], f16, tag="mt")
                c_sb = c_pool.tile([P, 4, K], f16, tag="csb")

                # DVE: z = (L > s) * q0
                z = z_w[:, :, :K]
                for s in range(4):
                    nc.vector.tensor_scalar(
                        out=z[:, s, :], in0=ct[:, 20, :], scalar1=float(s),
                        scalar2=None, op0=is_gt)
                nc.vector.tensor_tensor(out=z, in0=z, in1=ct[:, 0:4, :],
                                        op=mult)
                # DVE: M[4s+f] = z[s] * feats[s,f]
                zbc = z.unsqueeze(2).broadcast_to([P, 4, 4, K])
                fts = ct[:, 4:20, :].rearrange("p (s f) k -> p s f k", s=4)
                msf = mt.rearrange("p (s f) k -> p s f k", s=4)
                nc.vector.tensor_tensor(out=msf[:], in0=zbc, in1=fts,
                                        op=mult)

                # PE/Act: c[f] = sum_s M[s,f], per 128-sample group
                for g in range(K // GRP):
                    gs = slice(g * GRP, (g + 1) * GRP)
                    c_ps = psg_pool.tile([128, 4, GRP], f32, tag="cps")
                    for s in range(4):
                        nc.tensor.matmul(c_ps[:], ident[:],
                                         msf[:, s, :, gs],
                                         start=(s == 0), stop=(s == 3))
                    nc.scalar.copy(c_sb[:, :, gs], c_ps[:])

                # DVE: A-stage of the previous tile (pipeline)
                if pend is not None:
                    a_stage(*pend[:3])
                    nc.sync.dma_start(
                        out=y_h[:, :, pend[3]:pend[3] + pend[2]],
                        in_=pend[1][:])
                pend = (c_sb, yt, K, base)
                base += K

            a_stage(*pend[:3])
            nc.sync.dma_start(out=y_h[:, :, pend[3]:pend[3] + pend[2]],
                              in_=pend[1][:])
    nc.compile()
    return nc


_NC_CACHE = None


def _get_nc():
    global _NC_CACHE
    if _NC_CACHE is None:
        _NC_CACHE = build_nc()
    return _NC_CACHE


def _prep_core(args):
    xss, seq, c = args
    x = np.zeros((BSP, 4, 5), np.float16)
    x[:BS] = xss[c * BS:(c + 1) * BS]
    lp = np.zeros((BSP,), np.float16)
    lp[:BS] = seq[c * BS:(c + 1) * BS]
    arr = x.reshape(P, T, 4, 5)
    cin = np.empty((P, 21, T), np.float16)
    cin[:, 0:4] = arr[:, :, :, 0].transpose(0, 2, 1)
    cin[:, 4:20] = arr[:, :, :, 1:].transpose(0, 2, 3, 1).reshape(P, 16, T)
    cin[:, 20] = lp.reshape(P, T)
    return {"cin": cin.reshape(P, 21 * T)}


def _shard_inputs(xss, seq_lengths, W_kernel, W_reg):
    xss = np.asarray(xss, dtype=np.float32).reshape(B_TOTAL, 4, 5)
    seq = np.asarray(seq_lengths)
    wk = np.ascontiguousarray(W_kernel, dtype=np.float32)
    wr = np.ascontiguousarray(W_reg, dtype=np.float32)
    with ThreadPoolExecutor(N_CORES) as ex:
        maps = list(ex.map(_prep_core,
                           [(xss, seq, c) for c in range(N_CORES)]))
    for m in maps:
        m["w_kernel"] = wk
        m["w_reg"] = wr
    return maps


def run(xss, seq_lengths, W_kernel, W_reg, trace=False, **spmd_kwargs):
    nc = _get_nc()
    in_maps = _shard_inputs(xss, seq_lengths, W_kernel, W_reg)
    res = run_bass_kernel_spmd(nc, in_maps, core_ids=list(range(N_CORES)),
                               trace=trace, **spmd_kwargs)

    def _post(r):
        y = r["y"].reshape(P, 4, T).transpose(0, 2, 1).reshape(BSP, 4)
        return y[:BS].astype(np.float32)

    with ThreadPoolExecutor(N_CORES) as ex:
        parts = list(ex.map(_post, res.results))
    out = np.concatenate(parts, axis=0)
    return out, res


def kernel(xss, seq_lengths, W_kernel, W_reg):
    out, _ = run(xss, seq_lengths, W_kernel, W_reg)
    return out


# revision 18
# speedup vs baseline: 3.1325x; 1.0369x over previous
"""Trainium2 Bass kernel for nn_ContextualModel_75806172774985.

Per-sample computation (B = 4M samples, S=4 steps, Q=5 features):
    y[b, m] = sum_{s < L[b]} q0[b,s] * (A @ feats[b,s])[m],
    A = W_reg @ W_kernel  (4x4)

Memory-bound problem. Measured engine rates (fp16, per elem per partition):
DVE tensor_tensor 0.54ns (any AP shape), tensor_scalar 0.31ns,
GpSimd 1.69ns, Act 0.93ns; engines contend heavily when run concurrently,
so the design keeps DVE as the single SBUF-elementwise engine and moves
the s-summation to TensorE (PSUM traffic, not SBUF).

  - Host converts inputs to fp16, packs one dense per-partition stream:
        cin [P, 21, T]: rows 0-3  q0, rows 4-19 feats (row 4+4s+f),
                        row 20 seq_lengths. Output y [P, 4, T] fp16
        m-major; host transposes back / upcasts.
  - Per tile (software-pipelined one deep):
        DVE : zm[s] = (L > s)        4x tensor_scalar (imm)
              z    = zm * q0         1x tensor_tensor
              M[4s+f] = z[s]*f[s,f]  1x tensor_tensor (bcast over f)
        PE  : c = sum_s M[4s:4s+4]   4 accumulating fp16 identity matmuls
              per 128-sample group -> PSUM (f-major [4,128] per group)
        Act : c PSUM -> SBUF fp16 copy per group
        DVE : t[4m+f] = A[m,f]*c[f]  16x tensor_scalar (AP scalar)
              y[m] = sum_f t[4m+f]   2x slab tensor_tensor adds
    A is computed on device once (tiny ones-matmul broadcast into a_sb).
"""
import numpy as np
from concurrent.futures import ThreadPoolExecutor

import concourse.bass as bass
import concourse.tile as tile
from concourse import bacc, mybir
from concourse.bass_utils import run_bass_kernel_spmd

N_CORES = 8
P = 128
B_TOTAL = 4_000_000
BS = B_TOTAL // N_CORES          # 500_000 samples per core
T = 3968                         # samples per partition (128*3968 = 507904)
BSP = P * T
GRP = 128                        # samples per PSUM group (512 psum cols)

f32 = mybir.dt.float32
f16 = mybir.dt.float16

K_TILES = (128, 384, 896, 896, 896, 768)
ACT_F = (2, 3)                   # A-mul rows t[4m+f] for these f run on Act


def build_nc(num_devices=N_CORES):
    assert sum(K_TILES) == T
    for k in K_TILES:
        assert k % GRP == 0
    nc = bacc.Bacc("TRN2", target_bir_lowering=False, debug=False,
                   enable_asserts=False, num_devices=num_devices)

    c_d = nc.dram_tensor("cin", [P, 21 * T], f16, kind="ExternalInput")
    wk_d = nc.dram_tensor("w_kernel", [4, 4], f32, kind="ExternalInput")
    wr_d = nc.dram_tensor("w_reg", [4, 4], f32, kind="ExternalInput")
    y_d = nc.dram_tensor("y", [P, 4 * T], f16, kind="ExternalOutput")

    ones_np = np.ones((4, 128), dtype=np.float32)
    ones_d = nc.inline_tensor(ones_np, name="ones4x128")
    ident_np = np.eye(128, dtype=np.float16)
    ident_d = nc.inline_tensor(ident_np, name="ident128f16")

    c_h = c_d.ap().rearrange("p (r t) -> p r t", r=21)       # [128, 21, T]
    y_h = y_d.ap().rearrange("p (m t) -> p m t", m=4)        # [128, 4, T]

    mult = mybir.AluOpType.mult
    add = mybir.AluOpType.add
    is_gt = mybir.AluOpType.is_gt

    with tile.TileContext(nc) as tc:
        with (
            tc.tile_pool(name="cin", bufs=2) as in_pool,
            tc.tile_pool(name="mp", bufs=2) as m_pool,
            tc.tile_pool(name="cp", bufs=2) as c_pool,
            tc.tile_pool(name="work", bufs=1) as work,
            tc.tile_pool(name="yout", bufs=2) as y_pool,
            tc.tile_pool(name="singles", bufs=1) as singles,
            tc.tile_pool(name="ps", bufs=1, space="PSUM") as ps_pool,
            tc.tile_pool(name="psg", bufs=4, space="PSUM") as psg_pool,
        ):
            # ---- one-time: a_sb[p, 4f+m] = A[m,f] = sum_c Wreg[m,c]Wkern[c,f]
            wrT = singles.tile([4, 4], f32)
            nc.sync.dma_start(out=wrT[:], in_=wr_d.ap().transpose([1, 0]))
            wk_s = singles.tile([4, 4], f32)
            nc.sync.dma_start(out=wk_s[:], in_=wk_d.ap())
            ones_sb = singles.tile([4, 128], f32)
            nc.sync.dma_start(out=ones_sb[:], in_=ones_d.ap())
            ident = singles.tile([128, 128], f16)
            nc.sync.dma_start(out=ident[:], in_=ident_d.ap())

            wrT_rep = bass.AP(tensor=wrT.tensor, offset=wrT.offset,
                              ap=[list(wrT.ap[0]), [0, 4], [1, 4]])
            wk_rep = bass.AP(tensor=wk_s.tensor, offset=wk_s.offset,
                             ap=[list(wk_s.ap[0]), [1, 4], [0, 4]])
            r_sb = singles.tile([4, 4, 4], f32)
            nc.vector.tensor_tensor(out=r_sb[:], in0=wrT_rep, in1=wk_rep,
                                    op=mult)
            a_ps = ps_pool.tile([128, 16], f32)
            nc.tensor.matmul(a_ps[:], ones_sb[:],
                             r_sb.rearrange("c f m -> c (f m)"))
            a_sb = singles.tile([128, 16], f32)
            nc.scalar.copy(a_sb[:], a_ps[:])

            # ---- work tiles ----
            kmax = max(K_TILES)
            z_w = work.tile([P, 4, kmax], f16)
            t_w = work.tile([P, 16, kmax], f16)

            # ---- main loop, software-pipelined one tile deep ----
            pend = None              # (c_sb, yt, K, base) awaiting A-stage

            def a_stage(c_sb, yt, K):
                tk = t_w.rearrange("p (m f) k -> p m f k", m=4)[:, :, :, :K]
                for m in range(4):
                    for f in range(4):
                        if f in ACT_F:
                            nc.scalar.activation(
                                out=tk[:, m, f, :], in_=c_sb[:, f, :K],
                                func=mybir.ActivationFunctionType.Copy,
                                scale=a_sb[:, 4 * f + m:4 * f + m + 1])
                        else:
                            nc.vector.tensor_scalar(
                                out=tk[:, m, f, :], in0=c_sb[:, f, :K],
                                scalar1=a_sb[:, 4 * f + m:4 * f + m + 1],
                                scalar2=None, op0=mult)
                nc.vector.tensor_tensor(out=tk[:, :, 0:2], in0=tk[:, :, 0:2],
                                        in1=tk[:, :, 2:4], op=add)
                nc.vector.tensor_tensor(out=yt[:], in0=tk[:, :, 0],
                                        in1=tk[:, :, 1], op=add)

            base = 0
            for K in K_TILES:
                ct = in_pool.tile([P, 21, K], f16)
                nc.sync.dma_start(out=ct[:], in_=c_h[:, :, base:base + K])
                yt = y_pool.tile([P, 4, K], f16)
                mt = m_pool.tile([P, 16, K], f16, tag="mt")
                c_sb = c_pool.tile([P, 4, K], f16, tag="csb")

                # DVE: z = (L > s) * q0
                z = z_w[:, :, :K]
                for s in range(4):
                    nc.vector.tensor_scalar(
                        out=z[:, s, :], in0=ct[:, 20, :], scalar1=float(s),
                        scalar2=None, op0=is_gt)
                nc.vector.tensor_tensor(out=z, in0=z, in1=ct[:, 0:4, :],
                                        op=mult)
                # DVE: M[4s+f] = z[s] * feats[s,f]
                zbc = z.unsqueeze(2).broadcast_to([P, 4, 4, K])
                fts = ct[:, 4:20, :].rearrange("p (s f) k -> p s f k", s=4)
                msf = mt.rearrange("p (s f) k -> p s f k", s=4)
                nc.vector.tensor_tensor(out=msf[:], in0=zbc, in1=fts,
                                        op=mult)

                # PE/Act: c[f] = sum_s M[s,f], per 128-sample group
                for g in range(K // GRP):
                    gs = slice(g * GRP, (g + 1) * GRP)
                    c_ps = psg_pool.tile([128, 4, GRP], f32, tag="cps")
                    for s in range(4):
                        nc.tensor.matmul(c_ps[:], ident[:],
                                         msf[:, s, :, gs],
                                         start=(s == 0), stop=(s == 3))
                    nc.scalar.copy(c_sb[:, :, gs], c_ps[:])

                # DVE: A-stage of the previous tile (pipeline)
                if pend is not None:
                    a_stage(*pend[:3])
                    nc.sync.dma_start(
                        out=y_h[:, :, pend[3]:pend[3] + pend[2]],
                        in_=pend[1][:])
                pend = (c_sb, yt, K, base)
                base += K

            a_stage(*pend[:3])
            nc.sync.dma_start(out=y_h[:, :, pend[3]:pend[3] + pend[2]],
                              in_=pend[1][:])
    nc.compile()
    return nc


_NC_CACHE = None


def _get_nc():
    global _NC_CACHE
    if _NC_CACHE is None:
        _NC_CACHE = build_nc()
    return _NC_CACHE


def _prep_core(args):
    xss, seq, c = args
    x = np.zeros((BSP, 4, 5), np.float16)
    x[:BS] = xss[c * BS:(c + 1) * BS]
    lp = np.zeros((BSP,), np.float16)
    lp[:BS] = seq[c * BS:(c + 1) * BS]
    arr = x.reshape(P, T, 4, 5)
    cin = np.empty((P, 21, T), np.float16)
    cin[:, 0:4] = arr[:, :, :, 0].transpose(0, 2, 1)
    cin[:, 4:20] = arr[:, :, :, 1:].transpose(0, 2, 3, 1).reshape(P, 16, T)
    cin[:, 20] = lp.reshape(P, T)
    return {"cin": cin.reshape(P, 21 * T)}


def _shard_inputs(xss, seq_lengths, W_kernel, W_reg):
    xss = np.asarray(xss, dtype=np.float32).reshape(B_TOTAL, 4, 5)
    seq = np.asarray(seq_lengths)
    wk = np.ascontiguousarray(W_kernel, dtype=np.float32)
    wr = np.ascontiguousarray(W_reg, dtype=np.float32)
    with ThreadPoolExecutor(N_CORES) as ex:
        maps = list(ex.map(_prep_core,
                           [(xss, seq, c) for c in range(N_CORES)]))
    for m in maps:
        m["w_kernel"] = wk
        m["w_reg"] = wr
    return maps


def run(xss, seq_lengths, W_kernel, W_reg, trace=False, **spmd_kwargs):
    nc = _get_nc()
    in_maps = _shard_inputs(xss, seq_lengths, W_kernel, W_reg)
    res = run_bass_kernel_spmd(nc, in_maps, core_ids=list(range(N_CORES)),
                               trace=trace, **spmd_kwargs)

    def _post(r):
        y = r["y"].reshape(P, 4, T).transpose(0, 2, 1).reshape(BSP, 4)
        return y[:BS].astype(np.float32)

    with ThreadPoolExecutor(N_CORES) as ex:
        parts = list(ex.map(_post, res.results))
    out = np.concatenate(parts, axis=0)
    return out, res


def kernel(xss, seq_lengths, W_kernel, W_reg):
    out, _ = run(xss, seq_lengths, W_kernel, W_reg)
    return out


# revision 19
# speedup vs baseline: 3.1803x; 1.0153x over previous
"""Trainium2 Bass kernel for nn_ContextualModel_75806172774985.

Per-sample computation (B = 4M samples, S=4 steps, Q=5 features):
    y[b, m] = sum_{s < L[b]} q0[b,s] * (A @ feats[b,s])[m],
    A = W_reg @ W_kernel  (4x4)

Memory-bound problem. Measured engine rates (fp16, per elem per partition):
DVE tensor_tensor 0.54ns (any AP shape), tensor_scalar 0.31ns,
GpSimd 1.69ns, Act 0.93ns; engines contend heavily when run concurrently,
so the design keeps DVE as the single SBUF-elementwise engine and moves
the s-summation to TensorE (PSUM traffic, not SBUF).

  - Host converts inputs to fp16, packs one dense per-partition stream:
        cin [P, 21, T]: rows 0-3  q0, rows 4-19 feats (row 4+4s+f),
                        row 20 seq_lengths. Output y [P, 4, T] fp16
        m-major; host transposes back / upcasts.
  - Per tile (software-pipelined one deep):
        DVE : zm[s] = (L > s)        4x tensor_scalar (imm)
              z    = zm * q0         1x tensor_tensor
              M[4s+f] = z[s]*f[s,f]  1x tensor_tensor (bcast over f)
        PE  : c = sum_s M[4s:4s+4]   4 accumulating fp16 identity matmuls
              per 128-sample group -> PSUM (f-major [4,128] per group)
        Act : c PSUM -> SBUF fp16 copy per group
        DVE : t[4m+f] = A[m,f]*c[f]  16x tensor_scalar (AP scalar)
              y[m] = sum_f t[4m+f]   2x slab tensor_tensor adds
    A is computed on device once (tiny ones-matmul broadcast into a_sb).
"""
import numpy as np
from concurrent.futures import ThreadPoolExecutor

import concourse.bass as bass
import concourse.tile as tile
from concourse import bacc, mybir
from concourse.bass_utils import run_bass_kernel_spmd

N_CORES = 8
P = 128
B_TOTAL = 4_000_000
BS = B_TOTAL // N_CORES          # 500_000 samples per core
T = 3968                         # samples per partition (128*3968 = 507904)
BSP = P * T
GRP = 128                        # samples per PSUM group (512 psum cols)

f32 = mybir.dt.float32
f16 = mybir.dt.float16

K_TILES = (128, 384, 896, 896, 896, 768)
ACT_F = (2, 3)                   # A-mul rows t[4m+f] for these f run on Act


def build_nc(num_devices=N_CORES):
    assert sum(K_TILES) == T
    for k in K_TILES:
        assert k % GRP == 0
    nc = bacc.Bacc("TRN2", target_bir_lowering=False, debug=False,
                   enable_asserts=False, num_devices=num_devices)

    c_d = nc.dram_tensor("cin", [P, 21 * T], f16, kind="ExternalInput")
    wk_d = nc.dram_tensor("w_kernel", [4, 4], f32, kind="ExternalInput")
    wr_d = nc.dram_tensor("w_reg", [4, 4], f32, kind="ExternalInput")
    y_d = nc.dram_tensor("y", [P, 4 * T], f16, kind="ExternalOutput")

    ones_np = np.ones((4, 128), dtype=np.float32)
    ones_d = nc.inline_tensor(ones_np, name="ones4x128")
    ident_np = np.eye(128, dtype=np.float16)
    ident_d = nc.inline_tensor(ident_np, name="ident128f16")

    c_h = c_d.ap().rearrange("p (r t) -> p r t", r=21)       # [128, 21, T]
    y_h = y_d.ap().rearrange("p (m t) -> p m t", m=4)        # [128, 4, T]

    mult = mybir.AluOpType.mult
    add = mybir.AluOpType.add
    is_gt = mybir.AluOpType.is_gt

    with tile.TileContext(nc) as tc:
        with (
            tc.tile_pool(name="cin", bufs=2) as in_pool,
            tc.tile_pool(name="mp", bufs=2) as m_pool,
            tc.tile_pool(name="cp", bufs=3) as c_pool,
            tc.tile_pool(name="work", bufs=1) as work,
            tc.tile_pool(name="yout", bufs=2) as y_pool,
            tc.tile_pool(name="singles", bufs=1) as singles,
            tc.tile_pool(name="ps", bufs=1, space="PSUM") as ps_pool,
            tc.tile_pool(name="psg", bufs=4, space="PSUM") as psg_pool,
        ):
            # ---- one-time: a_sb[p, 4f+m] = A[m,f] = sum_c Wreg[m,c]Wkern[c,f]
            wrT = singles.tile([4, 4], f32)
            nc.sync.dma_start(out=wrT[:], in_=wr_d.ap().transpose([1, 0]))
            wk_s = singles.tile([4, 4], f32)
            nc.sync.dma_start(out=wk_s[:], in_=wk_d.ap())
            ones_sb = singles.tile([4, 128], f32)
            nc.sync.dma_start(out=ones_sb[:], in_=ones_d.ap())
            ident = singles.tile([128, 128], f16)
            nc.sync.dma_start(out=ident[:], in_=ident_d.ap())

            wrT_rep = bass.AP(tensor=wrT.tensor, offset=wrT.offset,
                              ap=[list(wrT.ap[0]), [0, 4], [1, 4]])
            wk_rep = bass.AP(tensor=wk_s.tensor, offset=wk_s.offset,
                             ap=[list(wk_s.ap[0]), [1, 4], [0, 4]])
            r_sb = singles.tile([4, 4, 4], f32)
            nc.vector.tensor_tensor(out=r_sb[:], in0=wrT_rep, in1=wk_rep,
                                    op=mult)
            a_ps = ps_pool.tile([128, 16], f32)
            nc.tensor.matmul(a_ps[:], ones_sb[:],
                             r_sb.rearrange("c f m -> c (f m)"))
            a_sb = singles.tile([128, 16], f32)
            nc.scalar.copy(a_sb[:], a_ps[:])

            # ---- work tiles ----
            kmax = max(K_TILES)
            z_w = work.tile([P, 4, kmax], f16)
            t_w = work.tile([P, 16, kmax], f16)

            # ---- main loop, software-pipelined one tile deep ----
            pend = None              # (c_sb, yt, K, base) awaiting A-stage

            def a_stage(c_sb, yt, K):
                tk = t_w.rearrange("p (m f) k -> p m f k", m=4)[:, :, :, :K]
                for m in range(4):
                    for f in range(4):
                        if f in ACT_F:
                            nc.scalar.activation(
                                out=tk[:, m, f, :], in_=c_sb[:, f, :K],
                                func=mybir.ActivationFunctionType.Copy,
                                scale=a_sb[:, 4 * f + m:4 * f + m + 1])
                        else:
                            nc.vector.tensor_scalar(
                                out=tk[:, m, f, :], in0=c_sb[:, f, :K],
                                scalar1=a_sb[:, 4 * f + m:4 * f + m + 1],
                                scalar2=None, op0=mult)
                nc.vector.tensor_tensor(out=tk[:, :, 0:2], in0=tk[:, :, 0:2],
                                        in1=tk[:, :, 2:4], op=add)
                nc.vector.tensor_tensor(out=yt[:], in0=tk[:, :, 0],
                                        in1=tk[:, :, 1], op=add)

            base = 0
            for K in K_TILES:
                ct = in_pool.tile([P, 21, K], f16)
                nc.sync.dma_start(out=ct[:], in_=c_h[:, :, base:base + K])
                yt = y_pool.tile([P, 4, K], f16)
                mt = m_pool.tile([P, 16, K], f16, tag="mt")
                c_sb = c_pool.tile([P, 4, K], f16, tag="csb")

                # DVE: z = (L > s) * q0
                z = z_w[:, :, :K]
                for s in range(4):
                    nc.vector.tensor_scalar(
                        out=z[:, s, :], in0=ct[:, 20, :], scalar1=float(s),
                        scalar2=None, op0=is_gt)
                nc.vector.tensor_tensor(out=z, in0=z, in1=ct[:, 0:4, :],
                                        op=mult)
                # DVE: M[4s+f] = z[s] * feats[s,f]
                zbc = z.unsqueeze(2).broadcast_to([P, 4, 4, K])
                fts = ct[:, 4:20, :].rearrange("p (s f) k -> p s f k", s=4)
                msf = mt.rearrange("p (s f) k -> p s f k", s=4)
                nc.vector.tensor_tensor(out=msf[:], in0=zbc, in1=fts,
                                        op=mult)

                # A-stage of the previous tile first: its Act rows must
                # not queue behind this tile's PSUM copies on Act
                if pend is not None:
                    a_stage(*pend[:3])
                    nc.sync.dma_start(
                        out=y_h[:, :, pend[3]:pend[3] + pend[2]],
                        in_=pend[1][:])

                # PE/Act: c[f] = sum_s M[s,f], per 128-sample group
                for g in range(K // GRP):
                    gs = slice(g * GRP, (g + 1) * GRP)
                    c_ps = psg_pool.tile([128, 4, GRP], f32, tag="cps")
                    for s in range(4):
                        nc.tensor.matmul(c_ps[:], ident[:],
                                         msf[:, s, :, gs],
                                         start=(s == 0), stop=(s == 3))
                    nc.scalar.copy(c_sb[:, :, gs], c_ps[:])

                pend = (c_sb, yt, K, base)
                base += K

            a_stage(*pend[:3])
            nc.sync.dma_start(out=y_h[:, :, pend[3]:pend[3] + pend[2]],
                              in_=pend[1][:])
    nc.compile()
    return nc


_NC_CACHE = None


def _get_nc():
    global _NC_CACHE
    if _NC_CACHE is None:
        _NC_CACHE = build_nc()
    return _NC_CACHE


def _prep_core(args):
    xss, seq, c = args
    x = np.zeros((BSP, 4, 5), np.float16)
    x[:BS] = xss[c * BS:(c + 1) * BS]
    lp = np.zeros((BSP,), np.float16)
    lp[:BS] = seq[c * BS:(c + 1) * BS]
    arr = x.reshape(P, T, 4, 5)
    cin = np.empty((P, 21, T), np.float16)
    cin[:, 0:4] = arr[:, :, :, 0].transpose(0, 2, 1)
    cin[:, 4:20] = arr[:, :, :, 1:].transpose(0, 2, 3, 1).reshape(P, 16, T)
    cin[:, 20] = lp.reshape(P, T)
    return {"cin": cin.reshape(P, 21 * T)}


def _shard_inputs(xss, seq_lengths, W_kernel, W_reg):
    xss = np.asarray(xss, dtype=np.float32).reshape(B_TOTAL, 4, 5)
    seq = np.asarray(seq_lengths)
    wk = np.ascontiguousarray(W_kernel, dtype=np.float32)
    wr = np.ascontiguousarray(W_reg, dtype=np.float32)
    with ThreadPoolExecutor(N_CORES) as ex:
        maps = list(ex.map(_prep_core,
                           [(xss, seq, c) for c in range(N_CORES)]))
    for m in maps:
        m["w_kernel"] = wk
        m["w_reg"] = wr
    return maps


def run(xss, seq_lengths, W_kernel, W_reg, trace=False, **spmd_kwargs):
    nc = _get_nc()
    in_maps = _shard_inputs(xss, seq_lengths, W_kernel, W_reg)
    res = run_bass_kernel_spmd(nc, in_maps, core_ids=list(range(N_CORES)),
                               trace=trace, **spmd_kwargs)

    def _post(r):
        y = r["y"].reshape(P, 4, T).transpose(0, 2, 1).reshape(BSP, 4)
        return y[:BS].astype(np.float32)

    with ThreadPoolExecutor(N_CORES) as ex:
        parts = list(ex.map(_post, res.results))
    out = np.concatenate(parts, axis=0)
    return out, res


def kernel(xss, seq_lengths, W_kernel, W_reg):
    out, _ = run(xss, seq_lengths, W_kernel, W_reg)
    return out


# revision 20
# speedup vs baseline: 3.2313x; 1.0160x over previous
"""Trainium2 Bass kernel for nn_ContextualModel_75806172774985.

Per-sample computation (B = 4M samples, S=4 steps, Q=5 features):
    y[b, m] = sum_{s < L[b]} q0[b,s] * (A @ feats[b,s])[m],
    A = W_reg @ W_kernel  (4x4)

Memory-bound problem. Measured engine rates (fp16, per elem per partition):
DVE tensor_tensor 0.54ns (any AP shape), tensor_scalar 0.31ns,
GpSimd 1.69ns, Act 0.93ns; engines contend heavily when run concurrently,
so the design keeps DVE as the single SBUF-elementwise engine and moves
the s-summation to TensorE (PSUM traffic, not SBUF).

  - Host converts inputs to fp16, packs one dense per-partition stream:
        cin [P, 21, T]: rows 0-3  q0, rows 4-19 feats (row 4+4s+f),
                        row 20 seq_lengths. Output y [P, 4, T] fp16
        m-major; host transposes back / upcasts.
  - Per tile (software-pipelined one deep):
        DVE : zm[s] = (L > s)        4x tensor_scalar (imm)
              z    = zm * q0         1x tensor_tensor
              M[4s+f] = z[s]*f[s,f]  1x tensor_tensor (bcast over f)
        PE  : c = sum_s M[4s:4s+4]   4 accumulating fp16 identity matmuls
              per 128-sample group -> PSUM (f-major [4,128] per group)
        Act : c PSUM -> SBUF fp16 copy per group
        DVE : t[4m+f] = A[m,f]*c[f]  16x tensor_scalar (AP scalar)
              y[m] = sum_f t[4m+f]   2x slab tensor_tensor adds
    A is computed on device once (tiny ones-matmul broadcast into a_sb).
"""
import numpy as np
from concurrent.futures import ThreadPoolExecutor

import concourse.bass as bass
import concourse.tile as tile
from concourse import bacc, mybir
from concourse.bass_utils import run_bass_kernel_spmd

N_CORES = 8
P = 128
B_TOTAL = 4_000_000
BS = B_TOTAL // N_CORES          # 500_000 samples per core
T = 3968                         # samples per partition (128*3968 = 507904)
BSP = P * T
GRP = 128                        # samples per PSUM group (512 psum cols)

f32 = mybir.dt.float32
f16 = mybir.dt.float16

K_TILES = (128, 384, 896, 896, 896, 768)
ACT_F = (2, 3)                   # A-mul rows t[4m+f] for these f run on Act


def build_nc(num_devices=N_CORES):
    assert sum(K_TILES) == T
    for k in K_TILES:
        assert k % GRP == 0
    nc = bacc.Bacc("TRN2", target_bir_lowering=False, debug=False,
                   enable_asserts=False, num_devices=num_devices)

    c_d = nc.dram_tensor("cin", [P, 21 * T], f16, kind="ExternalInput")
    wk_d = nc.dram_tensor("w_kernel", [4, 4], f32, kind="ExternalInput")
    wr_d = nc.dram_tensor("w_reg", [4, 4], f32, kind="ExternalInput")
    y_d = nc.dram_tensor("y", [P, 4 * T], f16, kind="ExternalOutput")

    ones_np = np.ones((4, 128), dtype=np.float32)
    ones_d = nc.inline_tensor(ones_np, name="ones4x128")
    ident_np = np.eye(128, dtype=np.float16)
    ident_d = nc.inline_tensor(ident_np, name="ident128f16")

    c_h = c_d.ap().rearrange("p (r t) -> p r t", r=21)       # [128, 21, T]
    y_h = y_d.ap().rearrange("p (m t) -> p m t", m=4)        # [128, 4, T]

    mult = mybir.AluOpType.mult
    add = mybir.AluOpType.add
    is_gt = mybir.AluOpType.is_gt

    with tile.TileContext(nc) as tc:
        with (
            tc.tile_pool(name="cin", bufs=2) as in_pool,
            tc.tile_pool(name="mp", bufs=2) as m_pool,
            tc.tile_pool(name="cp", bufs=3) as c_pool,
            tc.tile_pool(name="work", bufs=1) as work,
            tc.tile_pool(name="yout", bufs=2) as y_pool,
            tc.tile_pool(name="singles", bufs=1) as singles,
            tc.tile_pool(name="ps", bufs=1, space="PSUM") as ps_pool,
            tc.tile_pool(name="psg", bufs=4, space="PSUM") as psg_pool,
        ):
            # ---- one-time: a_sb[p, 4f+m] = A[m,f] = sum_c Wreg[m,c]Wkern[c,f]
            wrT = singles.tile([4, 4], f32)
            nc.sync.dma_start(out=wrT[:], in_=wr_d.ap().transpose([1, 0]))
            wk_s = singles.tile([4, 4], f32)
            nc.sync.dma_start(out=wk_s[:], in_=wk_d.ap())
            ones_sb = singles.tile([4, 128], f32)
            nc.sync.dma_start(out=ones_sb[:], in_=ones_d.ap())
            ident = singles.tile([128, 128], f16)
            nc.sync.dma_start(out=ident[:], in_=ident_d.ap())

            wrT_rep = bass.AP(tensor=wrT.tensor, offset=wrT.offset,
                              ap=[list(wrT.ap[0]), [0, 4], [1, 4]])
            wk_rep = bass.AP(tensor=wk_s.tensor, offset=wk_s.offset,
                             ap=[list(wk_s.ap[0]), [1, 4], [0, 4]])
            r_sb = singles.tile([4, 4, 4], f32)
            nc.vector.tensor_tensor(out=r_sb[:], in0=wrT_rep, in1=wk_rep,
                                    op=mult)
            a_ps = ps_pool.tile([128, 16], f32)
            nc.tensor.matmul(a_ps[:], ones_sb[:],
                             r_sb.rearrange("c f m -> c (f m)"))
            a_sb = singles.tile([128, 16], f32)
            nc.scalar.copy(a_sb[:], a_ps[:])

            # ---- work tiles ----
            kmax = max(K_TILES)
            z_w = work.tile([P, 4, kmax], f16)
            t_w = work.tile([P, 16, kmax], f16)

            # ---- main loop, software-pipelined one tile deep ----
            pend = None              # (c_sb, yt, K, base) awaiting A-stage

            def a_stage(c_sb, yt, K):
                tk = t_w.rearrange("p (m f) k -> p m f k", m=4)[:, :, :, :K]
                for m in range(4):
                    for f in range(4):
                        if f in ACT_F:
                            nc.scalar.activation(
                                out=tk[:, m, f, :], in_=c_sb[:, f, :K],
                                func=mybir.ActivationFunctionType.Copy,
                                scale=a_sb[:, 4 * f + m:4 * f + m + 1])
                        else:
                            nc.vector.tensor_scalar(
                                out=tk[:, m, f, :], in0=c_sb[:, f, :K],
                                scalar1=a_sb[:, 4 * f + m:4 * f + m + 1],
                                scalar2=None, op0=mult)
                nc.vector.tensor_tensor(out=tk[:, :, 0:2], in0=tk[:, :, 0:2],
                                        in1=tk[:, :, 2:4], op=add)
                nc.vector.tensor_tensor(out=yt[:], in0=tk[:, :, 0],
                                        in1=tk[:, :, 1], op=add)

            base = 0
            for K in K_TILES:
                ct = in_pool.tile([P, 21, K], f16)
                nc.sync.dma_start(out=ct[:], in_=c_h[:, :, base:base + K])
                yt = y_pool.tile([P, 4, K], f16)
                mt = m_pool.tile([P, 16, K], f16, tag="mt")
                c_sb = c_pool.tile([P, 4, K], f16, tag="csb")

                # DVE: z = (L > s) * q0
                z = z_w[:, :, :K]
                for s in range(4):
                    nc.vector.tensor_scalar(
                        out=z[:, s, :], in0=ct[:, 20, :], scalar1=float(s),
                        scalar2=None, op0=is_gt)
                nc.vector.tensor_tensor(out=z, in0=z, in1=ct[:, 0:4, :],
                                        op=mult)
                # DVE: M[4s+f] = z[s] * feats[s,f]
                fts = ct[:, 4:20, :].rearrange("p (s f) k -> p s f k", s=4)
                msf = mt.rearrange("p (s f) k -> p s f k", s=4)
                h = K // 2
                for lo, hi in ((0, h), (h, K)):
                    zbc = z[:, :, lo:hi].unsqueeze(2).broadcast_to(
                        [P, 4, 4, hi - lo])
                    nc.vector.tensor_tensor(out=msf[:, :, :, lo:hi],
                                            in0=zbc,
                                            in1=fts[:, :, :, lo:hi],
                                            op=mult)

                # A-stage of the previous tile first: its Act rows must
                # not queue behind this tile's PSUM copies on Act
                if pend is not None:
                    a_stage(*pend[:3])
                    nc.gpsimd.dma_start(
                        out=y_h[:, :, pend[3]:pend[3] + pend[2]],
                        in_=pend[1][:])

                # PE/Act: c[f] = sum_s M[s,f], per 128-sample group
                for g in range(K // GRP):
                    gs = slice(g * GRP, (g + 1) * GRP)
                    c_ps = psg_pool.tile([128, 4, GRP], f32, tag="cps")
                    for s in range(4):
                        nc.tensor.matmul(c_ps[:], ident[:],
                                         msf[:, s, :, gs],
                                         start=(s == 0), stop=(s == 3))
                    nc.scalar.copy(c_sb[:, :, gs], c_ps[:])

                pend = (c_sb, yt, K, base)
                base += K

            a_stage(*pend[:3])
            nc.gpsimd.dma_start(out=y_h[:, :, pend[3]:pend[3] + pend[2]],
                                in_=pend[1][:])
    nc.compile()
    return nc


_NC_CACHE = None


def _get_nc():
    global _NC_CACHE
    if _NC_CACHE is None:
        _NC_CACHE = build_nc()
    return _NC_CACHE


def _prep_core(args):
    xss, seq, c = args
    x = np.zeros((BSP, 4, 5), np.float16)
    x[:BS] = xss[c * BS:(c + 1) * BS]
    lp = np.zeros((BSP,), np.float16)
    lp[:BS] = seq[c * BS:(c + 1) * BS]
    arr = x.reshape(P, T, 4, 5)
    cin = np.empty((P, 21, T), np.float16)
    cin[:, 0:4] = arr[:, :, :, 0].transpose(0, 2, 1)
    cin[:, 4:20] = arr[:, :, :, 1:].transpose(0, 2, 3, 1).reshape(P, 16, T)
    cin[:, 20] = lp.reshape(P, T)
    return {"cin": cin.reshape(P, 21 * T)}


def _shard_inputs(xss, seq_lengths, W_kernel, W_reg):
    xss = np.asarray(xss, dtype=np.float32).reshape(B_TOTAL, 4, 5)
    seq = np.asarray(seq_lengths)
    wk = np.ascontiguousarray(W_kernel, dtype=np.float32)
    wr = np.ascontiguousarray(W_reg, dtype=np.float32)
    with ThreadPoolExecutor(N_CORES) as ex:
        maps = list(ex.map(_prep_core,
                           [(xss, seq, c) for c in range(N_CORES)]))
    for m in maps:
        m["w_kernel"] = wk
        m["w_reg"] = wr
    return maps


def run(xss, seq_lengths, W_kernel, W_reg, trace=False, **spmd_kwargs):
    nc = _get_nc()
    in_maps = _shard_inputs(xss, seq_lengths, W_kernel, W_reg)
    res = run_bass_kernel_spmd(nc, in_maps, core_ids=list(range(N_CORES)),
                               trace=trace, **spmd_kwargs)

    def _post(r):
        y = r["y"].reshape(P, 4, T).transpose(0, 2, 1).reshape(BSP, 4)
        return y[:BS].astype(np.float32)

    with ThreadPoolExecutor(N_CORES) as ex:
        parts = list(ex.map(_post, res.results))
    out = np.concatenate(parts, axis=0)
    return out, res


def kernel(xss, seq_lengths, W_kernel, W_reg):
    out, _ = run(xss, seq_lengths, W_kernel, W_reg)
    return out
